# revision 6
# baseline (speedup 1.0000x reference)
"""Trainium2 Bass kernel for the GNN message-passing attention layer.

Math (per site s = (b, l)):
    ac[k, fg, h]   (+= sa[fg, h] at k == 0)
    acb = ac * (beta[fg, h] + eps)
    fin0[k, h] = sum_fg gw[k, fg] * exp(acb[k, fg, h])
    fin[k, h]  = fin0[k, h] / (sum_k fin0[k, h] + eps)      # fin0 > 0, so
                                                            # abs == identity
    out[fo, h] = sum_k no[k, fo, h] * fin[k, h]

The reference subtracts max(acb) over (k, fg) before the exp; because the
normalizer divides it out again, skipping it only perturbs the eps term by
a ~1e-7 relative amount while exp stays comfortably inside fp32 range
(|acb| < ~40 for randn*randn inputs).

Sharding: the 8192 (b, l) sites are data-parallel; each of the 8 cores gets
1024 contiguous sites and processes them as 8 SBUF tiles of 128 sites
(sites on partitions, all reductions in the free dimension).
"""

import numpy as np

import concourse.bacc as bacc
import concourse.bass as bass
import concourse.mybir as mybir
import concourse.tile as tile
from concourse.bass_utils import run_bass_kernel_spmd

# Problem shapes (hardcoded; see module docstring).
B, L, K, Fg, H, Fo = 4, 2048, 16, 32, 4, 32
FgH = Fg * H  # 128
FoH = Fo * H  # 128
EPS = 1e-6

N_CORES = 8
SITES = B * L  # 8192
SPC = SITES // N_CORES  # 1024 sites per core
P = 128  # sites per SBUF tile (partition dim)
NT = SPC // P  # 8 tiles per core

F32 = mybir.dt.float32
ALU = mybir.AluOpType

# How many of the K=16 neighbor slices of the two big GPSIMD multiplies are
# offloaded to the vector engine instead (load-balance knobs).
B_DVE_K = 0  # slices of t = exp(acb) * gw done on DVE
E_DVE_K = 4  # slices of t2 = no * fin done on DVE


def tile_kernel(tc, outs, ins):
    """Build the per-core Tile program. outs = (out, fin); ins = (ac, no, gw,
    beta, sa), all DRAM APs shaped [SPC, ...] flattened per site."""
    nc = tc.nc
    out_d, fin_d = outs
    ac_d, no_d, gw_d, beta_d, sa_d = ins

    with (
        tc.tile_pool(name="big", bufs=3) as big,
        tc.tile_pool(name="small", bufs=3) as small,
    ):
        for i in range(NT):
            lo, hi = i * P, (i + 1) * P

            ac = big.tile([P, K * FgH], F32, tag="ac")
            no = big.tile([P, K * FoH], F32, tag="no")
            gw = small.tile([P, K * Fg], F32, tag="gw")
            beta = small.tile([P, FgH], F32, tag="beta")
            sa = small.tile([P, FgH], F32, tag="sa")
            fin0 = small.tile([P, K, H], F32, tag="fin0")
            s = small.tile([P, H], F32, tag="s")
            r = small.tile([P, H], F32, tag="r")
            out_t = small.tile([P, FoH], F32, tag="out_t")

            nc.sync.dma_start(out=ac, in_=ac_d[lo:hi])
            nc.sync.dma_start(out=no, in_=no_d[lo:hi])
            nc.sync.dma_start(out=gw, in_=gw_d[lo:hi])
            nc.sync.dma_start(out=beta, in_=beta_d[lo:hi])
            nc.sync.dma_start(out=sa, in_=sa_d[lo:hi])

            # Views.
            ac4 = ac.rearrange("p (k f h) -> p k f h", k=K, f=Fg)  # [P,K,Fg,H]
            ac3 = ac.rearrange("p (k d) -> p k d", k=K)  # [P,K,FgH]
            no4 = no.rearrange("p (k f h) -> p k f h", k=K, f=Fo)  # [P,K,Fo,H]
            gw3 = gw.rearrange("p (k f) -> p k f", k=K)  # [P,K,Fg]

            # ac[0] += sa (neighbor slot 0 gets the self-attention term).
            nc.vector.tensor_add(out=ac[:, 0:FgH], in0=ac[:, 0:FgH], in1=sa)

            # acb = (beta + eps) * ac, beta broadcast over K.  One DVE pass.
            beta_b = beta.unsqueeze(1).broadcast_to([P, K, FgH])
            nc.vector.scalar_tensor_tensor(
                out=ac3, in0=beta_b, scalar=EPS, in1=ac3, op0=ALU.add, op1=ALU.mult
            )

            # e = exp(acb) on the scalar (ACT) engine, in place.
            nc.scalar.activation(out=ac, in_=ac, func=mybir.ActivationFunctionType.Exp)

            # t = e * gw, gw broadcast over H.  Split K slices POOL/DVE.
            gw_b = gw3.unsqueeze(3).broadcast_to([P, K, Fg, H])
            kb = K - B_DVE_K
            if kb > 0:
                nc.gpsimd.tensor_mul(
                    out=ac4[:, :kb], in0=ac4[:, :kb], in1=gw_b[:, :kb]
                )
            if B_DVE_K > 0:
                nc.vector.tensor_mul(
                    out=ac4[:, kb:], in0=ac4[:, kb:], in1=gw_b[:, kb:]
                )

            # fin0[k, h] = sum_fg t[k, fg, h]  (reduce innermost axis of the
            # transposed view [P, K, H, Fg]).
            nc.vector.reduce_sum(
                out=fin0, in_=ac4.transpose([0, 1, 3, 2]), axis=mybir.AxisListType.X
            )

            # s[h] = sum_k fin0[k, h]; r = 1 / (s + eps).
            nc.vector.reduce_sum(
                out=s, in_=fin0.transpose([0, 2, 1]), axis=mybir.AxisListType.X
            )
            nc.vector.tensor_scalar_add(out=s, in0=s, scalar1=EPS)
            nc.vector.reciprocal(out=r, in_=s)

            # fin = fin0 * r  (r broadcast over K).  This is an output.
            r_b = r.unsqueeze(1).broadcast_to([P, K, H])
            nc.vector.tensor_mul(out=fin0, in0=fin0, in1=r_b)
            nc.sync.dma_start(out=fin_d[lo:hi], in_=fin0)

            # t2 = no * fin (fin broadcast over Fo).  Split K slices POOL/DVE.
            fin_b = fin0.unsqueeze(2).broadcast_to([P, K, Fo, H])
            ke = K - E_DVE_K
            if ke > 0:
                nc.gpsimd.tensor_mul(
                    out=no4[:, :ke], in0=no4[:, :ke], in1=fin_b[:, :ke]
                )
            if E_DVE_K > 0:
                nc.vector.tensor_mul(
                    out=no4[:, ke:], in0=no4[:, ke:], in1=fin_b[:, ke:]
                )

            # out[foh] = sum_k t2[k, foh]  (reduce innermost axis of the
            # transposed view [P, FoH, K]).
            no_t = no.rearrange("p (k d) -> p k d", k=K).transpose([0, 2, 1])
            nc.vector.reduce_sum(out=out_t, in_=no_t, axis=mybir.AxisListType.X)
            nc.sync.dma_start(out=out_d[lo:hi], in_=out_t)


def build_bass():
    nc = bacc.Bacc(
        "TRN2",
        target_bir_lowering=False,
        debug=False,
        num_devices=N_CORES,
    )
    ins = (
        nc.dram_tensor("ac", [SPC, K * FgH], F32, kind="ExternalInput").ap(),
        nc.dram_tensor("no", [SPC, K * FoH], F32, kind="ExternalInput").ap(),
        nc.dram_tensor("gw", [SPC, K * Fg], F32, kind="ExternalInput").ap(),
        nc.dram_tensor("beta", [SPC, FgH], F32, kind="ExternalInput").ap(),
        nc.dram_tensor("sa", [SPC, FgH], F32, kind="ExternalInput").ap(),
    )
    outs = (
        nc.dram_tensor("out", [SPC, FoH], F32, kind="ExternalOutput").ap(),
        nc.dram_tensor("fin", [SPC, K * H], F32, kind="ExternalOutput").ap(),
    )
    with tile.TileContext(nc) as tc:
        tile_kernel(tc, outs, ins)
    nc.compile()
    return nc


LAST_RESULTS = None  # BassKernelResults from the most recent kernel() call


def kernel(
    beta,
    self_attention,
    attention_coefficients,
    node_outputs,
    graph_weights,
    _trace=False,
):
    beta = np.ascontiguousarray(np.asarray(beta, np.float32)).reshape(SITES, FgH)
    sa = np.ascontiguousarray(np.asarray(self_attention, np.float32)).reshape(
        SITES, FgH
    )
    ac = np.ascontiguousarray(np.asarray(attention_coefficients, np.float32)).reshape(
        SITES, K * FgH
    )
    no = np.ascontiguousarray(np.asarray(node_outputs, np.float32)).reshape(
        SITES, K * FoH
    )
    gw = np.ascontiguousarray(np.asarray(graph_weights, np.float32)).reshape(
        SITES, K * Fg
    )

    in_maps = []
    for c in range(N_CORES):
        sl = slice(c * SPC, (c + 1) * SPC)
        in_maps.append(
            {
                "ac": np.ascontiguousarray(ac[sl]),
                "no": np.ascontiguousarray(no[sl]),
                "gw": np.ascontiguousarray(gw[sl]),
                "beta": np.ascontiguousarray(beta[sl]),
                "sa": np.ascontiguousarray(sa[sl]),
            }
        )

    nc = build_bass()
    res = run_bass_kernel_spmd(
        nc, in_maps, core_ids=list(range(N_CORES)), trace=_trace
    )
    global LAST_RESULTS
    LAST_RESULTS = res

    out = np.concatenate([r["out"] for r in res.results], axis=0).reshape(B, L, FoH)
    fin = np.concatenate([r["fin"] for r in res.results], axis=0).reshape(B, L, K, H)
    return out, fin


# revision 10
# speedup vs baseline: 1.0848x; 1.0848x over previous
"""Trainium2 Bass kernel for the GNN message-passing attention layer.

Math (per site s = (b, l)):
    ac[k, fg, h]   (+= sa[fg, h] at k == 0)
    acb = ac * (beta[fg, h] + eps)
    fin0[k, h] = sum_fg gw[k, fg] * exp(acb[k, fg, h])
    fin[k, h]  = fin0[k, h] / (sum_k fin0[k, h] + eps)      # fin0 > 0, so
                                                            # abs == identity
    out[fo, h] = sum_k no[k, fo, h] * fin[k, h]

The reference subtracts max(acb) over (k, fg) before the exp; because the
normalizer divides it out again, skipping it only perturbs the eps term by
a ~1e-7 relative amount while exp stays comfortably inside fp32 range
(|acb| < ~40 for randn*randn inputs).

Sharding: the 8192 (b, l) sites are data-parallel; each of the 8 cores gets
1024 contiguous sites and processes them as 8 SBUF tiles of 128 sites
(sites on partitions, all reductions in the free dimension).

Engine plan per 128-site tile (measured ns on HW):
    ACT : betaeps = beta + eps (300), exp (2400), two PSUM->SBUF copies (500)
    POOL: A = ac * betaeps_b (3820), part of E = no * fin_b
    DVE : B = e * gw_b (2360), C = sum_fg via 5 tree-halving adds (2770),
          normalizer smalls, rest of E
    PE  : F = sum_k via 16 PSUM-accumulated identity matmuls + transpose back
    DMA : all loads/stores + sa added into ac slot 0 via SWDGE CCE accum-add
"""

import numpy as np

import concourse.bacc as bacc
import concourse.bass as bass
import concourse.mybir as mybir
import concourse.tile as tile
from concourse.bass_utils import run_bass_kernel_spmd

# Problem shapes (hardcoded; see module docstring).
B, L, K, Fg, H, Fo = 4, 2048, 16, 32, 4, 32
FgH = Fg * H  # 128
FoH = Fo * H  # 128
EPS = 1e-6

N_CORES = 8
SITES = B * L  # 8192
SPC = SITES // N_CORES  # 1024 sites per core
P = 128  # sites per SBUF tile (partition dim)
NT = SPC // P  # 8 tiles per core

F32 = mybir.dt.float32
ALU = mybir.AluOpType
AXL = mybir.AxisListType
AF = mybir.ActivationFunctionType

# ---- engine-assignment knobs (tuned against the HW profile) ----
A_ON_POOL = True  # A = ac * betaeps: POOL TT (else DVE stt with fused eps)
B_ON_POOL = False  # B = e * gw_b
C_R1_ON_POOL = False  # first (1024-elem) C tree round on POOL
E_DVE_K = 4  # how many of E's K=16 slices go on DVE (rest POOL)
F_MODE = "pe"  # "pe" (PSUM-accumulated matmuls) or "tree" (DVE adds)
SA_VIA_DMA = True  # sa added into ac slot 0 by SWDGE accum DMA
BUFS = 4


def tile_kernel(tc, outs, ins):
    """Build the per-core Tile program. outs = (out, fin); ins = (ac, no, gw,
    beta, sa, ident), all DRAM APs shaped [SPC, ...] flattened per site."""
    nc = tc.nc
    out_d, fin_d = outs
    ac_d, no_d, gw_d, beta_d, sa_d, ident_d = ins

    with (
        tc.tile_pool(name="big", bufs=BUFS) as big,
        tc.tile_pool(name="small", bufs=BUFS) as small,
        tc.tile_pool(name="singles", bufs=1) as singles,
        tc.tile_pool(name="psum", bufs=2, space="PSUM") as psum,
    ):
        ident = singles.tile([P, P], F32)
        nc.sync.dma_start(out=ident, in_=ident_d)
        eps_t = singles.tile([P, 1], F32)
        nc.vector.memset(eps_t, EPS)

        for i in range(NT):
            lo, hi = i * P, (i + 1) * P

            ac = big.tile([P, K * FgH], F32, tag="ac")
            no = big.tile([P, K * FoH], F32, tag="no")
            gw = small.tile([P, K * Fg], F32, tag="gw")
            beta = small.tile([P, FgH], F32, tag="beta")
            betaeps = small.tile([P, FgH], F32, tag="betaeps")
            sa = small.tile([P, FgH], F32, tag="sa")
            fin0 = small.tile([P, K, H], F32, tag="fin0")
            s = small.tile([P, H], F32, tag="s")
            r = small.tile([P, H], F32, tag="r")
            u1 = small.tile([P, P], F32, tag="u1")
            out_t = small.tile([P, FoH], F32, tag="out_t")

            nc.sync.dma_start(out=ac, in_=ac_d[lo:hi])
            nc.sync.dma_start(out=no, in_=no_d[lo:hi])
            nc.sync.dma_start(out=gw, in_=gw_d[lo:hi])
            nc.sync.dma_start(out=beta, in_=beta_d[lo:hi])

            # ac[0] += sa (self-attention into neighbor slot 0).
            if SA_VIA_DMA:
                nc.gpsimd.dma_start(
                    out=ac[:, 0:FgH], in_=sa_d[lo:hi], accum_op=ALU.add
                )
            else:
                nc.sync.dma_start(out=sa, in_=sa_d[lo:hi])
                nc.vector.tensor_add(out=ac[:, 0:FgH], in0=ac[:, 0:FgH], in1=sa)

            # Views.
            ac4 = ac.rearrange("p (k f h) -> p k f h", k=K, f=Fg)  # [P,K,Fg,H]
            ac3 = ac.rearrange("p (k d) -> p k d", k=K)  # [P,K,FgH]
            no4 = no.rearrange("p (k f h) -> p k f h", k=K, f=Fo)  # [P,K,Fo,H]
            gw3 = gw.rearrange("p (k f) -> p k f", k=K)  # [P,K,Fg]

            # A: acb = (beta + eps) * ac, beta broadcast over K (in place).
            beta_b = beta.unsqueeze(1).broadcast_to([P, K, FgH])
            if A_ON_POOL:
                nc.scalar.activation(
                    out=betaeps, in_=beta, func=AF.Identity, bias=eps_t
                )
                be_b = betaeps.unsqueeze(1).broadcast_to([P, K, FgH])
                nc.gpsimd.tensor_mul(out=ac3, in0=ac3, in1=be_b)
            else:
                nc.vector.scalar_tensor_tensor(
                    out=ac3, in0=beta_b, scalar=EPS, in1=ac3, op0=ALU.add, op1=ALU.mult
                )

            # e = exp(acb) on ACT, in place.
            nc.scalar.activation(out=ac, in_=ac, func=AF.Exp)

            # B: t = e * gw (gw broadcast over H), in place.
            gw_b = gw3.unsqueeze(3).broadcast_to([P, K, Fg, H])
            if B_ON_POOL:
                nc.gpsimd.tensor_mul(out=ac4, in0=ac4, in1=gw_b)
            else:
                nc.vector.tensor_mul(out=ac4, in0=ac4, in1=gw_b)

            # C: fin0[k, h] = sum_fg t[k, fg, h] via tree halving (in place,
            # last round writes the compact fin0 tile).
            r1 = nc.gpsimd if C_R1_ON_POOL else nc.vector
            r1.tensor_add(
                out=ac4[:, :, 0:16], in0=ac4[:, :, 0:16], in1=ac4[:, :, 16:32]
            )
            nc.vector.tensor_add(
                out=ac4[:, :, 0:8], in0=ac4[:, :, 0:8], in1=ac4[:, :, 8:16]
            )
            nc.vector.tensor_add(
                out=ac4[:, :, 0:4], in0=ac4[:, :, 0:4], in1=ac4[:, :, 4:8]
            )
            nc.vector.tensor_add(
                out=ac4[:, :, 0:2], in0=ac4[:, :, 0:2], in1=ac4[:, :, 2:4]
            )
            nc.vector.tensor_add(
                out=fin0, in0=ac4[:, :, 0, :], in1=ac4[:, :, 1, :]
            )

            # s[h] = sum_k fin0[k, h]; r = 1 / (s + eps).
            nc.vector.reduce_sum(out=s, in_=fin0.transpose([0, 2, 1]), axis=AXL.X)
            nc.vector.tensor_scalar_add(out=s, in0=s, scalar1=EPS)
            nc.vector.reciprocal(out=r, in_=s)

            # fin = fin0 * r (r broadcast over K) — an output.
            r_b = r.unsqueeze(1).broadcast_to([P, K, H])
            nc.vector.tensor_mul(out=fin0, in0=fin0, in1=r_b)
            nc.sync.dma_start(out=fin_d[lo:hi], in_=fin0)

            # E: t2 = no * fin (fin broadcast over Fo), in place, split K
            # slices POOL/DVE.
            fin_b = fin0.unsqueeze(2).broadcast_to([P, K, Fo, H])
            ke = K - E_DVE_K
            if ke > 0:
                nc.gpsimd.tensor_mul(
                    out=no4[:, :ke], in0=no4[:, :ke], in1=fin_b[:, :ke]
                )
            if E_DVE_K > 0:
                nc.vector.tensor_mul(
                    out=no4[:, ke:], in0=no4[:, ke:], in1=fin_b[:, ke:]
                )

            # F: out[foh] = sum_k t2[k, foh].
            if F_MODE == "pe":
                # 16 identity matmuls accumulate transposed k-blocks into
                # PSUM: p1[foh, s] = sum_k t2[s, k*128 + foh]; then transpose
                # back through the PE and copy out.
                p1 = psum.tile([P, P], F32, tag="p1")
                p2 = psum.tile([P, P], F32, tag="p2")
                for k in range(K):
                    nc.tensor.matmul(
                        p1,
                        no[:, k * FoH : (k + 1) * FoH],
                        ident,
                        start=(k == 0),
                        stop=(k == K - 1),
                    )
                nc.scalar.copy(out=u1, in_=p1)
                nc.tensor.matmul(p2, u1, ident, start=True, stop=True)
                nc.scalar.copy(out=out_t, in_=p2)
            else:
                no3 = no.rearrange("p (k d) -> p k d", k=K)
                nc.vector.tensor_add(
                    out=no3[:, 0:8], in0=no3[:, 0:8], in1=no3[:, 8:16]
                )
                nc.vector.tensor_add(
                    out=no3[:, 0:4], in0=no3[:, 0:4], in1=no3[:, 4:8]
                )
                nc.vector.tensor_add(
                    out=no3[:, 0:2], in0=no3[:, 0:2], in1=no3[:, 2:4]
                )
                nc.vector.tensor_add(
                    out=out_t, in0=no3[:, 0], in1=no3[:, 1]
                )
            nc.sync.dma_start(out=out_d[lo:hi], in_=out_t)


def build_bass():
    nc = bacc.Bacc(
        "TRN2",
        target_bir_lowering=False,
        debug=False,
        num_devices=N_CORES,
    )
    ins = (
        nc.dram_tensor("ac", [SPC, K * FgH], F32, kind="ExternalInput").ap(),
        nc.dram_tensor("no", [SPC, K * FoH], F32, kind="ExternalInput").ap(),
        nc.dram_tensor("gw", [SPC, K * Fg], F32, kind="ExternalInput").ap(),
        nc.dram_tensor("beta", [SPC, FgH], F32, kind="ExternalInput").ap(),
        nc.dram_tensor("sa", [SPC, FgH], F32, kind="ExternalInput").ap(),
        nc.dram_tensor("ident", [P, P], F32, kind="ExternalInput").ap(),
    )
    outs = (
        nc.dram_tensor("out", [SPC, FoH], F32, kind="ExternalOutput").ap(),
        nc.dram_tensor("fin", [SPC, K * H], F32, kind="ExternalOutput").ap(),
    )
    with tile.TileContext(nc) as tc:
        tile_kernel(tc, outs, ins)
    nc.compile()
    return nc


LAST_RESULTS = None  # BassKernelResults from the most recent kernel() call


def kernel(
    beta,
    self_attention,
    attention_coefficients,
    node_outputs,
    graph_weights,
    _trace=False,
):
    beta = np.ascontiguousarray(np.asarray(beta, np.float32)).reshape(SITES, FgH)
    sa = np.ascontiguousarray(np.asarray(self_attention, np.float32)).reshape(
        SITES, FgH
    )
    ac = np.ascontiguousarray(np.asarray(attention_coefficients, np.float32)).reshape(
        SITES, K * FgH
    )
    no = np.ascontiguousarray(np.asarray(node_outputs, np.float32)).reshape(
        SITES, K * FoH
    )
    gw = np.ascontiguousarray(np.asarray(graph_weights, np.float32)).reshape(
        SITES, K * Fg
    )
    ident = np.eye(P, dtype=np.float32)

    in_maps = []
    for c in range(N_CORES):
        sl = slice(c * SPC, (c + 1) * SPC)
        in_maps.append(
            {
                "ac": np.ascontiguousarray(ac[sl]),
                "no": np.ascontiguousarray(no[sl]),
                "gw": np.ascontiguousarray(gw[sl]),
                "beta": np.ascontiguousarray(beta[sl]),
                "sa": np.ascontiguousarray(sa[sl]),
                "ident": ident,
            }
        )

    nc = build_bass()
    res = run_bass_kernel_spmd(
        nc, in_maps, core_ids=list(range(N_CORES)), trace=_trace
    )
    global LAST_RESULTS
    LAST_RESULTS = res

    out = np.concatenate([r["out"] for r in res.results], axis=0).reshape(B, L, FoH)
    fin = np.concatenate([r["fin"] for r in res.results], axis=0).reshape(B, L, K, H)
    return out, fin
